# revision 5
# baseline (speedup 1.0000x reference)
"""Trainium2 Bass kernel for the 2-layer LSTM decoder (B=128, T=32, F=2048,
E=512, H=1024, V=10000) — MODEL-PARALLEL over H across 8 NeuronCores.

Each core owns a 128-wide H-chunk of BOTH layers and computes it for the
FULL batch (M=128 matmuls, full PE utilization; per-core weight streaming
is 1/8 of the data-parallel scheme). Per step, ONE 8-core AllGather of
[h0T(t), h1T(t-1)] mixes the hidden state; h-chunks are transposed BEFORE
the gather so the AG output lands directly in matmul-lhsT layout
[K=h (8x128 part), M=b]. The vocab-sharded FC (1250 cols/core) runs
inside the loop two steps behind, filling PE idle during each AllGather.

MPDT=f8 (default): fp8e4m3 DoubleRow matmuls (2 k-chunks per
instruction), fp8 h-exchange. Power-of-2 scaling keeps fp8 operands in
the normal range: h x32, weights x64, PSUM carries x2048, descale is
folded into the ACT sigmoid/tanh scale and the FC drain. c-state, PSUM,
and gate activations stay fp32. MPDT=f16: plain fp16 operands.

AG index a: AG_0 = [h0T(init), h1T(init)]; AG_{t+1} = [h0T(t), h1T(t-1)]
for t=0..31; AG_33 = [h1T(31), h1T(31)]. Iteration a (=1..33) consumes
g_a: L1(a-1) uses slot0=h0T(a-1) + slot1=h1T(a-2); L0(a) uses slot0;
FC(a-2) uses slot1.

DMA queues: g-readback is split across both HWDGE queues (SP + ACT);
the FC output (fp16) rides the ACT queue behind the g-half.
"""

import os

import numpy as np

import concourse.bass as bass
import concourse.mybir as mybir
from concourse import bacc
from concourse.bass_utils import run_bass_kernel_spmd
from concourse.masks import make_identity
from concourse.tile import TileContext

P = 128
NCORES = 8
B, T, F, E, H, L, V = 128, 32, 2048, 512, 1024, 2, 10000
TB = T * B                # 4096 rows (t-major: row = t*128 + b)
HC = H // NCORES          # 128 h-cols per core per layer
GC = 4 * HC               # 512 gate cols per core per layer
VC = V // NCORES          # 1250 vocab cols per core
KF, KE, KH = F // P, E // P, H // P      # 16, 4, 8
NAG = T + 2               # 34 allgathers
F16 = mybir.dt.float16
F32 = mybir.dt.float32
F8 = mybir.dt.float8e4

MODE = "f16"
DT = F8 if MODE == "f8" else F16
DR = mybir.MatmulPerfMode.DoubleRow if MODE == "f8" else None
KSTEP = 2 if MODE == "f8" else 1
SA = 32.0 if MODE == "f8" else 1.0       # h / activation scale
SW = 64.0 if MODE == "f8" else 1.0       # recurrent + fc weight scale
SE = 64.0 if MODE == "f8" else 1.0       # embedding scale
SW0 = 32.0 if MODE == "f8" else 1.0      # wih0 scale (SE*SW0 == SA*SW)
SCALE = SA * SW                          # PSUM carries SCALE * true value
INV = 1.0 / SCALE

SIG = mybir.ActivationFunctionType.Sigmoid
TANH = mybir.ActivationFunctionType.Tanh
MUL = mybir.AluOpType.mult
ADD = mybir.AluOpType.add

_cache = {}


def _build_nc():
    nc = bacc.Bacc("TRN2", target_bir_lowering=False, debug=False,
                   enable_asserts=False, num_devices=NCORES)

    def din(name, shape, dt=DT):
        return nc.dram_tensor(name, shape, dt, kind="ExternalInput").ap()

    featT = din("featT", [F, B], F16)     # features.T, full batch (shared)
    emb_idx = din("emb_idx", [TB, 1], mybir.dt.int32)
    table = din("table", [V, E], F16)     # x SE
    initw = din("initw", [F, GC], F16)    # cols: h0|h1|c0|c1 for chunk
    initb = din("initb", [1, GC], F16)
    wih0T = din("wih0T", [E, GC])         # x SW0
    bsum0 = din("bsum0", [1, GC])         # x SCALE
    whh0T = din("whh0T", [H, GC])         # x SW
    whh1T = din("whh1T", [H, GC])         # x SW
    wih1T = din("wih1T", [H, GC])         # x SW
    bsum1 = din("bsum1", [1, GC])         # x SCALE
    fcwT = din("fcwT", [H, VC])           # x SW
    fcb_rep = din("fcb_rep", [P, VC], F32)

    out = nc.dram_tensor("out", [TB, VC], F16, kind="ExternalOutput").ap()

    agin = [nc.dram_tensor(f"agin{a}", [P, 2 * HC], DT, kind="Internal").ap()
            for a in range(NAG)]
    agout = [nc.dram_tensor(f"agout{a}", [NCORES * P, 2 * HC], DT,
                            kind="Internal", addr_space="Shared").ap()
             for a in range(NAG)]
    groups = [list(range(NCORES))]

    featT_v = featT.rearrange("(k p) b -> p k b", p=P)
    initw_v = initw.rearrange("(k p) n -> p k n", p=P)
    wih0T_v = wih0T.rearrange("(k p) n -> p k n", p=P)
    whh0T_v = whh0T.rearrange("(k p) n -> p k n", p=P)
    whh1T_v = whh1T.rearrange("(k p) n -> p k n", p=P)
    wih1T_v = wih1T.rearrange("(k p) n -> p k n", p=P)
    fcwT_v = fcwT.rearrange("(k p) n -> p k n", p=P)
    idx_v = emb_idx.rearrange("(g p) one -> p g one", p=P)
    agout_v = [a.rearrange("(j p) c -> p j c", p=P) for a in agout]

    with TileContext(nc) as tc:
        with tc.tile_pool(name="const", bufs=1) as constp, \
             tc.tile_pool(name="resident", bufs=1) as resp, \
             tc.tile_pool(name="state", bufs=1) as statep:

            id_dt = constp.tile([P, P], DT)
            make_identity(nc, id_dt)
            id16 = constp.tile([P, P], F16)
            make_identity(nc, id16)
            ones_dt = constp.tile([1, P], DT)
            nc.gpsimd.memset(ones_dt, 1.0)
            ones16 = constp.tile([1, P], F16)
            nc.gpsimd.memset(ones16, 1.0)

            # X0-prep weights first on SP (small; phase B + first emit_x0
            # gate the AllGather chain), recurrent weights behind them,
            # bulky FC weights on the ACT queue so they never delay phase B
            wih0_s = resp.tile([P, KE, GC], DT)
            nc.sync.dma_start(wih0_s, wih0T_v)
            bsum0_s = constp.tile([1, GC], DT)
            nc.sync.dma_start(bsum0_s, bsum0)
            fcw_s = resp.tile([P, KH, VC], DT)
            nc.scalar.dma_start(fcw_s, fcwT_v)
            fcb_s = resp.tile([P, VC], F32)
            nc.scalar.dma_start(fcb_s, fcb_rep)

            # Long-lived state
            c0_s = statep.tile([P, HC], F32)
            c1_s = statep.tile([P, HC], F32)
            h1Ti_s = statep.tile([P, HC], DT)    # init h1T chunk for AG_1
            X0_s = statep.tile([P, T, GC], DT)   # x SCALE

            # ---------------- Phase B: h0/c0/h1/c1 init -------------------
            with tc.tile_pool(name="initp", bufs=1) as initp, \
                 tc.tile_pool(name="initps", bufs=1, space="PSUM") as initps, \
                 tc.tile_pool(name="trps0", bufs=1, space="PSUM") as trps0:
                featT_s = initp.tile([P, KF, B], F16, tag="ft")
                nc.sync.dma_start(featT_s, featT_v)
                initw_s = initp.tile([P, KF, GC], F16, tag="iw")
                nc.sync.dma_start(initw_s, initw_v)
                initb_s = initp.tile([1, GC], F16, tag="ib")
                nc.sync.dma_start(initb_s, initb)
                ps = initps.tile([P, GC], F32)
                for k in range(KF):
                    nc.tensor.matmul(ps, featT_s[:, k, :], initw_s[:, k, :],
                                     start=(k == 0), stop=False)
                nc.tensor.matmul(ps, ones16, initb_s, start=False, stop=True)
                nc.vector.tensor_copy(c0_s, ps[:, 2 * HC : 3 * HC])
                nc.vector.tensor_copy(c1_s, ps[:, 3 * HC : 4 * HC])
                hh = initp.tile([P, 2 * HC], F16, tag="hh")
                nc.vector.tensor_scalar_mul(hh, ps[:, 0 : 2 * HC], SA)
                pt = trps0.tile([P, 2, HC], F16)
                nc.tensor.transpose(pt[:, 0, :], hh[:, 0:HC], id16)
                nc.tensor.transpose(pt[:, 1, :], hh[:, HC : 2 * HC], id16)
                nc.vector.tensor_copy(h1Ti_s, pt[:, 1, :])
                ag_sb = initp.tile([P, 2 * HC], DT, tag="ag0")
                nc.vector.tensor_copy(ag_sb, pt)
                nc.sync.dma_start(agin[0], ag_sb)
                nc.gpsimd.collective_compute(
                    "AllGather", mybir.AluOpType.bypass, replica_groups=groups,
                    ins=[agin[0].bitcast(F16)[:]],
                    outs=[agout[0].bitcast(F16)[:]])


            # recurrent weights: needed from L0(0) on, load under AG_0
            whh0_s = resp.tile([P, KH, GC], DT)
            nc.sync.dma_start(whh0_s, whh0T_v)
            whh1_s = resp.tile([P, KH, GC], DT)
            nc.sync.dma_start(whh1_s, whh1T_v)
            wih1_s = resp.tile([P, KH, GC], DT)
            nc.sync.dma_start(wih1_s, wih1T_v)
            bsum1_s = constp.tile([1, GC], DT)
            nc.sync.dma_start(bsum1_s, bsum1)

            # ---------------- Phase D: recurrence + fused FC --------------
            with tc.tile_pool(name="gp", bufs=2) as gpool, \
                 tc.tile_pool(name="embp", bufs=2) as embp, \
                 tc.tile_pool(name="aginp", bufs=2) as aginp, \
                 tc.tile_pool(name="ewp", bufs=2) as ewp, \
                 tc.tile_pool(name="outp", bufs=2) as outp, \
                 tc.tile_pool(name="gps", bufs=2, space="PSUM") as gps, \
                 tc.tile_pool(name="fcps", bufs=2, space="PSUM") as fcps, \
                 tc.tile_pool(name="trps", bufs=2, space="PSUM") as trps:

                def ew(ps, c_s):
                    """gates [i|f|o|g] (x SCALE) -> h chunk (x SA, DT)."""
                    sig = ewp.tile([P, 3 * HC], F32, tag="sig")
                    nc.scalar.activation(sig, ps[:, 0 : 3 * HC], SIG, scale=INV)
                    tg = ewp.tile([P, HC], F32, tag="tg")
                    nc.scalar.activation(tg, ps[:, 3 * HC : 4 * HC], TANH,
                                         scale=INV)
                    nc.vector.tensor_mul(c_s, sig[:, HC : 2 * HC], c_s)
                    tmp = ewp.tile([P, HC], F32, tag="tmp")
                    nc.vector.tensor_mul(tmp, sig[:, 0:HC], tg)
                    nc.vector.tensor_add(c_s, c_s, tmp)
                    tc_t = ewp.tile([P, HC], F32, tag="tc")
                    nc.scalar.activation(tc_t, c_s, TANH)
                    h_sb = ewp.tile([P, HC], F16, tag="h")
                    if MODE == "f8":
                        nc.vector.scalar_tensor_tensor(
                            h_sb, tc_t, SA, sig[:, 2 * HC : 3 * HC], MUL, MUL)
                    else:
                        nc.vector.tensor_mul(h_sb, sig[:, 2 * HC : 3 * HC],
                                             tc_t)
                    return h_sb

                def transpose_h(h_sb):
                    pt = trps.tile([P, HC], F16, tag="pt")
                    nc.tensor.transpose(pt, h_sb, id16)
                    return pt

                def l0_matmuls(g, t):
                    ps0 = gps.tile([P, GC], F32, tag="g0")
                    for k in range(0, KH, KSTEP):
                        nc.tensor.matmul(ps0, g[:, k : k + KSTEP, 0:HC],
                                         whh0_s[:, k : k + KSTEP, :],
                                         start=(k == 0), stop=False,
                                         perf_mode=DR)
                    nc.tensor.matmul(ps0, id_dt, X0_s[:, t, :],
                                     start=False, stop=True)
                    return ps0

                def l1_matmuls(g):
                    ps1 = gps.tile([P, GC], F32, tag="g1")
                    for k in range(0, KH, KSTEP):
                        nc.tensor.matmul(ps1, g[:, k : k + KSTEP, HC : 2 * HC],
                                         whh1_s[:, k : k + KSTEP, :],
                                         start=(k == 0), stop=False,
                                         perf_mode=DR)
                    for k in range(0, KH, KSTEP):
                        nc.tensor.matmul(ps1, g[:, k : k + KSTEP, 0:HC],
                                         wih1_s[:, k : k + KSTEP, :],
                                         start=False, stop=False, perf_mode=DR)
                    nc.tensor.matmul(ps1, ones_dt, bsum1_s,
                                     start=False, stop=True)
                    return ps1

                def fc_piece(g, pc, w):
                    ps = fcps.tile([P, 512], F32, tag="fc")
                    for k in range(0, KH, KSTEP):
                        nc.tensor.matmul(ps[:, :w],
                                         g[:, k : k + KSTEP, HC : 2 * HC],
                                         fcw_s[:, k : k + KSTEP, pc : pc + w],
                                         start=(k == 0),
                                         stop=(k + KSTEP >= KH), perf_mode=DR)
                    return ps

                def fc_drain(o_sb, piece):
                    (pc, w), ps = piece
                    nc.vector.scalar_tensor_tensor(
                        o_sb[:, pc : pc + w], ps[:, :w], INV,
                        fcb_s[:, pc : pc + w], MUL, ADD)

                FC_PIECES = []
                pc = 0
                while pc < VC:
                    w = min(512, VC - pc)
                    FC_PIECES.append((pc, w))
                    pc += w

                def emit_x0(t):
                    idx_t = embp.tile([P, 1, 1], mybir.dt.int32, tag="idx")
                    nc.sync.dma_start(idx_t, idx_v[:, t : t + 1, :])
                    rows = embp.tile([P, E], F16, tag="rows")
                    nc.gpsimd.indirect_dma_start(
                        out=rows[:], out_offset=None, in_=table[:],
                        in_offset=bass.IndirectOffsetOnAxis(
                            ap=idx_t[:, 0, :], axis=0))
                    ptE = trps.tile([P, KE, P], F16, tag="pt")
                    for ke in range(KE):
                        nc.tensor.transpose(
                            ptE[:, ke, :], rows[:, ke * P : (ke + 1) * P],
                            id16)
                    et = embp.tile([P, KE, P], DT, tag="et")
                    nc.vector.tensor_copy(et, ptE)
                    ps = fcps.tile([P, GC], F32, tag="fc")
                    for k in range(0, KE, KSTEP):
                        nc.tensor.matmul(
                            ps, et[:, k : k + KSTEP, :],
                            wih0_s[:, k : k + KSTEP, :],
                            start=(k == 0), stop=False, perf_mode=DR)
                    nc.tensor.matmul(ps, ones_dt, bsum0_s,
                                     start=False, stop=True)
                    nc.vector.tensor_copy(X0_s[:, t, :], ps)

                def load_g(a):
                    # per-k-chunk DMAs alternating across both HWDGE queues:
                    # subtile deps let matmuls start on chunk k as soon as
                    # its 64KB lands (~1us) instead of waiting for the full
                    # 512KB readback
                    g = gpool.tile([P, KH, 2 * HC], DT, tag="g")
                    for q in range(4):
                        eng = nc.sync if q % 2 == 0 else nc.scalar
                        eng.dma_start(g[:, 2 * q : 2 * q + 2, :],
                                      agout_v[a][:, 2 * q : 2 * q + 2, :])
                    return g

                def send_ag(a, pt0, pt1):
                    ag_sb = aginp.tile([P, 2 * HC], DT, tag="ag")
                    nc.vector.tensor_copy(ag_sb[:, 0:HC], pt0)
                    nc.vector.tensor_copy(ag_sb[:, HC : 2 * HC], pt1)
                    nc.sync.dma_start(agin[a], ag_sb)
                    nc.gpsimd.collective_compute(
                        "AllGather", mybir.AluOpType.bypass,
                        replica_groups=groups,
                        ins=[agin[a].bitcast(F16)[:]],
                        outs=[agout[a].bitcast(F16)[:]])

                # pre-loop: X0[0], X0[1]; L0(0) from g_0 (init AG)
                emit_x0(0)
                emit_x0(1)
                g_prev = load_g(0)
                ps0 = l0_matmuls(g_prev, 0)
                h0_sb = ew(ps0, c0_s)
                pt0 = transpose_h(h0_sb)
                send_ag(1, pt0, h1Ti_s)

                for a in range(1, NAG):
                    g = load_g(a)
                    # L1(a-1): whh1*h1(a-2) + wih1*h0(a-1) + b1
                    if a - 1 <= T - 1:
                        ps1 = l1_matmuls(g)
                    # L0(a): whh0*h0(a-1) + X0[a]
                    if a <= T - 1:
                        ps0 = l0_matmuls(g, a)
                    # elementwise tails (ACT/DVE run under following PE work)
                    pt1 = None
                    if a - 1 <= T - 1:
                        h1_sb = ew(ps1, c1_s)
                        pt1 = transpose_h(h1_sb)
                    # FC(a-2) piece 0 keeps PE busy while ew0 finishes
                    t_fc = a - 2
                    if 0 <= t_fc:
                        o_sb = outp.tile([P, VC], F16, tag="o")
                        prev = (FC_PIECES[0], fc_piece(g, *FC_PIECES[0]))
                    if a <= T - 1:
                        h0_sb = ew(ps0, c0_s)
                        pt0 = transpose_h(h0_sb)
                        send_ag(a + 1, pt0, pt1)
                    elif a == T:
                        # AG_33: only slot1 (h1T(31)) is consumed
                        send_ag(a + 1, pt1, pt1)
                    if 0 <= t_fc:
                        # remaining FC pieces run during the AllGather;
                        # drains interleave so the 2-deep PSUM ring never
                        # stalls the PE
                        for pc, w in FC_PIECES[1:]:
                            ps_n = fc_piece(g, pc, w)
                            fc_drain(o_sb, prev)
                            prev = ((pc, w), ps_n)
                        fc_drain(o_sb, prev)
                        nc.scalar.dma_start(
                            out[t_fc * P : (t_fc + 1) * P, :], o_sb)
                    if a + 1 <= T - 1:
                        emit_x0(a + 1)

    nc.finalize()
    return nc


def _get_compiled():
    if "nc" not in _cache:
        _cache["nc"] = _build_nc()
    return _cache["nc"]


def _to_dt(x, scale=1.0):
    x = np.asarray(x, np.float32) * scale
    if MODE == "f8":
        x = np.clip(x, -440.0, 440.0)
        return np.ascontiguousarray(x).astype(mybir.dt.np(F8))
    return np.ascontiguousarray(x).astype(np.float16)


def _prep_inputs(features, captions, embed_table, init_h_w, init_h_b,
                 init_c_w, init_c_b, w_ih0, w_hh0, b_ih0, b_hh0,
                 w_ih1, w_hh1, b_ih1, b_hh1, fc_w, fc_b):
    f16 = lambda x: np.ascontiguousarray(
        np.asarray(x), dtype=np.float32).astype(np.float16)
    fT16 = lambda x: np.ascontiguousarray(
        np.asarray(x, np.float32).T).astype(np.float16)
    tT = lambda x, s: _to_dt(np.asarray(x, np.float32).T, s)

    features = np.asarray(features, np.float32)
    captions = np.asarray(captions).astype(np.int32)
    w_ih0, w_hh0 = np.asarray(w_ih0, np.float32), np.asarray(w_hh0, np.float32)
    w_ih1, w_hh1 = np.asarray(w_ih1, np.float32), np.asarray(w_hh1, np.float32)
    bs0 = np.asarray(b_ih0, np.float32) + np.asarray(b_hh0, np.float32)
    bs1 = np.asarray(b_ih1, np.float32) + np.asarray(b_hh1, np.float32)
    init_h_w = np.asarray(init_h_w, np.float32)
    init_c_w = np.asarray(init_c_w, np.float32)
    init_h_b = np.asarray(init_h_b, np.float32)
    init_c_b = np.asarray(init_c_b, np.float32)
    fc_w = np.asarray(fc_w, np.float32)
    fc_b = np.asarray(fc_b, np.float32)

    shared = {
        "featT": fT16(features),
        "emb_idx": np.ascontiguousarray(captions.T.reshape(TB, 1)),
        "table": _to_dt(embed_table, SE),
    }

    in_maps = []
    for c in range(NCORES):
        hc = np.arange(c * HC, (c + 1) * HC)
        # gate piece order [i, f, o, g]; torch row blocks are i, f, g, o
        sel = np.concatenate([0 * H + hc, 1 * H + hc, 3 * H + hc, 2 * H + hc])
        vc = slice(c * VC, (c + 1) * VC)
        m = dict(shared)
        m["whh0T"] = tT(w_hh0[sel], SW)
        m["whh1T"] = tT(w_hh1[sel], SW)
        m["wih1T"] = tT(w_ih1[sel], SW)
        m["wih0T"] = tT(w_ih0[sel], SW0)
        m["bsum0"] = _to_dt(bs0[sel][None, :], SCALE)
        m["bsum1"] = _to_dt(bs1[sel][None, :], SCALE)
        # init_hidden rows: h0 -> h*L+0, h1 -> h*L+1 (reshape(B,H,L) perm)
        m["initw"] = fT16(np.concatenate(
            [init_h_w[2 * hc], init_h_w[2 * hc + 1],
             init_c_w[2 * hc], init_c_w[2 * hc + 1]], axis=0))
        m["initb"] = f16(np.concatenate(
            [init_h_b[2 * hc], init_h_b[2 * hc + 1],
             init_c_b[2 * hc], init_c_b[2 * hc + 1]])[None, :])
        m["fcwT"] = tT(fc_w[vc], SW)
        m["fcb_rep"] = np.ascontiguousarray(
            np.broadcast_to(fc_b[vc], (P, VC)), dtype=np.float32)
        in_maps.append(m)
    return in_maps


last_results = None


def kernel(**inputs) -> np.ndarray:
    global last_results
    nc = _get_compiled()
    in_maps = _prep_inputs(**inputs)
    res = run_bass_kernel_spmd(nc, in_maps, core_ids=list(range(NCORES)))
    last_results = res
    parts = [res.results[c]["out"].astype(np.float32) for c in range(NCORES)]
    return np.concatenate(parts, axis=1).reshape(T, B, V)
